# revision 3
# baseline (speedup 1.0000x reference)
"""Distributed Trainium2 Bass kernel for GQA attention prefill.

Problem: B=2, S=2048, D=4096, 32 q heads, 8 kv heads, head_dim=128, RoPE,
causal mask, start_pos=0.

Sharding (8 cores): DP2 over batch x TP4 over heads.  Core c = b*4 + g gets
batch b, q-heads 8g..8g+7, kv-heads 2g..2g+1, wo rows for those q-heads.
Each core computes a partial [S, D] output; the host sums the 4 partials
per batch (the row-parallel wo unshard).

On-core dataflow:
  x (f32) --cast DMA--> x_bf16 DRAM --DMA-transpose--> xT sbuf tiles
  QKV projection (bf16 matmuls, head-dim columns pre-permuted [even|odd])
  RoPE applied on the projection PSUM (cross-partition DVE ops)
  scoresT[t,s] = K^T.T @ Q^T, causal mask via on-chip affine_select tile,
  exp on ACT (no max subtraction; scores are ~N(0,1)),
  outT += V.T @ P^T accumulated over T-chunks, rowsum via ones-matmul,
  normalize, project with wo (bf16, streamed), DMA partial out.
"""

import math

import numpy as np

import concourse.bass as bass  # noqa: F401  (bass types via bacc)
import concourse.mybir as mybir
from concourse import bacc
from concourse.bass_utils import run_bass_kernel_spmd
from concourse.tile import TileContext

F32 = mybir.dt.float32
BF16 = mybir.dt.bfloat16

B, S, D = 2, 2048, 4096
NH, NKV, HD = 32, 8, 128
NCORES = 8
TPG = 4                  # tensor-parallel groups
NQL = NH // TPG          # 8 local q heads
NKVL = NKV // TPG        # 2 local kv heads
SCW = 512                # s-chunk width
NSC = S // SCW           # 4 s-chunks
NKC = D // 128           # 32 contraction chunks for projections
NTC = S // 128           # 16 T-chunks (key positions)
SCALE = 1.0 / math.sqrt(HD)
NEG = -1e9


def _build():
    nc = bacc.Bacc("TRN2", target_bir_lowering=False, debug=False,
                   num_devices=NCORES)
    x = nc.declare_dram_parameter("x", [S, D], F32, isOutput=False)
    wq = nc.declare_dram_parameter("wq", [D, NQL * HD], F32, isOutput=False)
    wk = nc.declare_dram_parameter("wk", [D, NKVL * HD], F32, isOutput=False)
    wv = nc.declare_dram_parameter("wv", [D, NKVL * HD], F32, isOutput=False)
    wo = nc.declare_dram_parameter("wo", [NQL * HD, D], F32, isOutput=False)
    cos = nc.declare_dram_parameter("cos", [S, 64], F32, isOutput=False)
    sin = nc.declare_dram_parameter("sin", [S, 64], F32, isOutput=False)
    out = nc.declare_dram_parameter("out", [S, D], F32, isOutput=True)

    NM = NQL + 2 * NKVL

    with TileContext(nc) as tc:
        with (
            tc.tile_pool(name="dram", bufs=1, space="DRAM") as dram,
            tc.tile_pool(name="const", bufs=1) as const,
            tc.tile_pool(name="big", bufs=1) as big,
            tc.tile_pool(name="sb", bufs=3) as sb,
            tc.tile_pool(name="ps", bufs=1, space="PSUM") as ps,
        ):
            xb = [dram.tile([SCW, D], BF16, name=f"xb{i}") for i in range(NSC)]
            for i in range(NSC):
                for j in range(4):
                    nc.gpsimd.dma_start(
                        out=xb[i][j * 128 : (j + 1) * 128, :],
                        in_=x[i * SCW + j * 128 : i * SCW + (j + 1) * 128, :],
                    )
            wqb = dram.tile([D, NQL * HD], BF16, name="wqb")
            wkb = dram.tile([D, NKVL * HD], BF16, name="wkb")
            wvb = dram.tile([D, NKVL * HD], BF16, name="wvb")
            wob = dram.tile([NQL * HD, D], BF16, name="wob")
            for j in range(4):
                sl = slice(j * (D // 4), (j + 1) * (D // 4))
                nc.gpsimd.dma_start(out=wqb[sl, :], in_=wq[sl, :])
                nc.gpsimd.dma_start(out=wkb[sl, :], in_=wk[sl, :])
                nc.gpsimd.dma_start(out=wvb[sl, :], in_=wv[sl, :])
            for j in range(4):
                sl = slice(j * (NQL * HD // 4), (j + 1) * (NQL * HD // 4))
                nc.gpsimd.dma_start(out=wob[sl, :], in_=wo[sl, :])

            ident = const.tile([128, 128], BF16, name="ident")
            nc.gpsimd.memset(ident[:], 0.0)
            nc.gpsimd.affine_select(
                out=ident[:], in_=ident[:],
                compare_op=mybir.AluOpType.not_equal, fill=1.0,
                base=0, pattern=[[-1, 128]], channel_multiplier=1,
            )
            ones = const.tile([128, 128], BF16, name="ones")
            nc.gpsimd.memset(ones[:], 1.0)
            maskbig = const.tile([128, 896], F32, name="maskbig")
            nc.gpsimd.memset(maskbig[:], 0.0)
            nc.gpsimd.affine_select(
                out=maskbig[:], in_=maskbig[:],
                compare_op=mybir.AluOpType.is_ge, fill=NEG,
                base=-384, pattern=[[1, 896]], channel_multiplier=-1,
            )
            cos2 = const.tile([128, S], BF16, name="cos2")
            sin2n = const.tile([128, S], BF16, name="sin2n")
            for i in range(S // 128):
                ct = sb.tile([128, 64], BF16, name=f"ct{i}", tag="ct")
                st = sb.tile([128, 64], BF16, name=f"st{i}", tag="st")
                nc.gpsimd.dma_start(out=ct[:], in_=cos[i * 128 : (i + 1) * 128, :])
                nc.gpsimd.dma_start(out=st[:], in_=sin[i * 128 : (i + 1) * 128, :])
                pc = ps.tile([128, 128], BF16, name=f"pc{i}", tag="tp")
                nc.tensor.transpose(pc[0:64, :], ct[:], ident[:])
                sl = slice(i * 128, (i + 1) * 128)
                nc.scalar.copy(out=cos2[0:64, sl], in_=pc[0:64, :])
                nc.scalar.copy(out=cos2[64:128, sl], in_=pc[0:64, :])
                pst = ps.tile([128, 128], BF16, name=f"pst{i}", tag="tp")
                nc.tensor.transpose(pst[0:64, :], st[:], ident[:])
                nc.scalar.mul(out=sin2n[0:64, sl], in_=pst[0:64, :], mul=-1.0)
                nc.scalar.copy(out=sin2n[64:128, sl], in_=pst[0:64, :])

            ksb = big.tile([128, NKVL * S], BF16, name="ksb")
            vsb = big.tile([128, NTC * NKVL * HD], BF16, name="vsb")

            for sc in range(NSC):
                ssl = slice(sc * SCW, (sc + 1) * SCW)
                xt = []
                for kc in range(NKC):
                    t = sb.tile([128, SCW], BF16, name=f"xt{sc}_{kc}",
                                tag="xt", bufs=NKC + 4)
                    nc.sync.dma_start(
                        out=t[:],
                        in_=xb[sc][:, kc * 128 : (kc + 1) * 128],
                        transpose=True,
                    )
                    xt.append(t)

                qtiles = []
                for m in range(NM):
                    wsl = sb.tile([128, NKC * 128], BF16, name=f"w{sc}_{m}",
                                  tag="wsl", bufs=3)
                    if m < NQL:
                        src = wqb[:, m * HD : (m + 1) * HD]
                    elif m < NQL + NKVL:
                        src = wkb[:, (m - NQL) * HD : (m - NQL + 1) * HD]
                    else:
                        kv = m - NQL - NKVL
                        src = wvb[:, kv * HD : (kv + 1) * HD]
                    nc.sync.dma_start(
                        out=wsl[:].rearrange("p (kc c) -> p kc c", kc=NKC),
                        in_=src.rearrange("(kc p) c -> p kc c", p=128),
                    )
                    pp = ps.tile([128, SCW], F32, name=f"pp{sc}_{m}",
                                 tag="proj", bufs=2)
                    for kc in range(NKC):
                        nc.tensor.matmul(
                            pp[:], wsl[:, kc * 128 : (kc + 1) * 128], xt[kc][:],
                            start=(kc == 0), stop=(kc == NKC - 1),
                        )
                    if m < NQL + NKVL:
                        if m < NQL:
                            qt = sb.tile([128, SCW], BF16, name=f"q{sc}_{m}",
                                         tag=f"q{m}", bufs=1)
                            dst = qt[:]
                            qtiles.append(qt)
                        else:
                            kv = m - NQL
                            dst = ksb[:, kv * S + sc * SCW : kv * S + (sc + 1) * SCW]
                        t1 = sb.tile([128, SCW], F32, name=f"t1_{sc}_{m}",
                                     tag="t1", bufs=2)
                        t2 = sb.tile([128, SCW], F32, name=f"t2_{sc}_{m}",
                                     tag="t2", bufs=2)
                        nc.vector.tensor_tensor(
                            out=t1[0:64, :], in0=pp[64:128, :],
                            in1=sin2n[0:64, ssl], op=mybir.AluOpType.mult)
                        nc.vector.tensor_tensor(
                            out=t1[64:128, :], in0=pp[0:64, :],
                            in1=sin2n[64:128, ssl], op=mybir.AluOpType.mult)
                        nc.vector.tensor_tensor(
                            out=t2[:], in0=pp[:], in1=cos2[:, ssl],
                            op=mybir.AluOpType.mult)
                        nc.vector.tensor_tensor(
                            out=dst, in0=t1[:], in1=t2[:],
                            op=mybir.AluOpType.add)
                    else:
                        kv = m - NQL - NKVL
                        vts = sb.tile([128, SCW], BF16, name=f"vts{sc}_{kv}",
                                      tag="vts", bufs=2)
                        nc.vector.tensor_copy(out=vts[:], in_=pp[:])
                        for j in range(SCW // 128):
                            pv = ps.tile([128, 128], BF16,
                                         name=f"pv{sc}_{kv}_{j}", tag="tp")
                            nc.tensor.transpose(
                                pv[:], vts[:, j * 128 : (j + 1) * 128], ident[:])
                            slot = (sc * 4 + j) * NKVL + kv
                            nc.scalar.copy(
                                out=vsb[:, slot * HD : (slot + 1) * HD], in_=pv[:])

                # ---- attention for this s-chunk ------------------------
                attnT = []
                ntc = 4 * sc + 4
                for h in range(NQL):
                    kv = h // (NQL // NKVL)
                    po = ps.tile([128, SCW], F32, name=f"po{sc}_{h}", tag="o")
                    pr = ps.tile([128, SCW], F32, name=f"pr{sc}_{h}", tag="r")
                    for tcx in range(ntc):
                        pss = ps.tile([128, SCW], F32,
                                      name=f"ps{sc}_{h}_{tcx}", tag="sc", bufs=2)
                        nc.tensor.matmul(
                            pss[:],
                            ksb[:, kv * S + tcx * 128 : kv * S + (tcx + 1) * 128],
                            qtiles[h][:],
                            start=True, stop=True,
                        )
                        if tcx >= 4 * sc:
                            dlt = 128 * (tcx - 4 * sc)
                            nc.vector.tensor_tensor(
                                out=pss[:], in0=pss[:],
                                in1=maskbig[:, 384 - dlt : 896 - dlt],
                                op=mybir.AluOpType.add)
                        pt = sb.tile([128, SCW], BF16, name=f"pt{sc}_{h}_{tcx}",
                                     tag="pt", bufs=4)
                        nc.scalar.activation(
                            pt[:], pss[:],
                            mybir.ActivationFunctionType.Exp, scale=SCALE)
                        slot = tcx * NKVL + kv
                        nc.tensor.matmul(
                            po[:], vsb[:, slot * HD : (slot + 1) * HD], pt[:],
                            start=(tcx == 0), stop=(tcx == ntc - 1))
                        nc.tensor.matmul(
                            pr[:], ones[:], pt[:],
                            start=(tcx == 0), stop=(tcx == ntc - 1))
                    rec = sb.tile([128, SCW], F32, name=f"rec{sc}_{h}",
                                  tag="rec", bufs=2)
                    nc.vector.reciprocal(rec[:], pr[:])
                    at = sb.tile([128, SCW], BF16, name=f"at{sc}_{h}",
                                 tag=f"at{h}", bufs=1)
                    nc.vector.tensor_tensor(
                        out=at[:], in0=po[:], in1=rec[:],
                        op=mybir.AluOpType.mult)
                    attnT.append(at)

                # ---- output projection (partial) -----------------------
                for dc in range(D // SCW):
                    wot = sb.tile([128, NQL * SCW], BF16, name=f"wot{sc}_{dc}",
                                  tag="wot", bufs=2)
                    nc.sync.dma_start(
                        out=wot[:].rearrange("p (kc c) -> p kc c", kc=NQL),
                        in_=wob[:, dc * SCW : (dc + 1) * SCW].rearrange(
                            "(kc p) c -> p kc c", p=128),
                    )
                    for ssub in range(SCW // 128):
                        pd = ps.tile([128, SCW], F32, name=f"pd{sc}_{dc}_{ssub}",
                                     tag="proj", bufs=2)
                        for kc8 in range(NQL):
                            nc.tensor.matmul(
                                pd[:],
                                attnT[kc8][:, ssub * 128 : (ssub + 1) * 128],
                                wot[:, kc8 * SCW : (kc8 + 1) * SCW],
                                start=(kc8 == 0), stop=(kc8 == NQL - 1))
                        os_ = sb.tile([128, SCW], F32, name=f"os{sc}_{dc}_{ssub}",
                                      tag="os", bufs=3)
                        nc.scalar.copy(out=os_[:], in_=pd[:])
                        nc.sync.dma_start(
                            out=out[sc * SCW + ssub * 128 : sc * SCW + (ssub + 1) * 128,
                                    dc * SCW : (dc + 1) * SCW],
                            in_=os_[:])
    nc.finalize()
    return nc


_NC_CACHE = None


def _get_graph():
    global _NC_CACHE
    if _NC_CACHE is None:
        _NC_CACHE = _build()
    return _NC_CACHE


_PERM = np.concatenate([np.arange(0, HD, 2), np.arange(1, HD, 2)])


def _shard_inputs(x, freqs_cos, freqs_sin, wq, wk, wv, wo):
    """Build the 8 per-core input maps (pure numpy slicing/permutation)."""
    x = np.ascontiguousarray(np.asarray(x, dtype=np.float32))
    wq = np.asarray(wq, dtype=np.float32)
    wk = np.asarray(wk, dtype=np.float32)
    wv = np.asarray(wv, dtype=np.float32)
    wo = np.asarray(wo, dtype=np.float32)
    cos = np.ascontiguousarray(np.asarray(freqs_cos, dtype=np.float32))
    sin = np.ascontiguousarray(np.asarray(freqs_sin, dtype=np.float32))

    wq4 = wq.reshape(D, NH, HD)
    wk4 = wk.reshape(D, NKV, HD)
    wv4 = wv.reshape(D, NKV, HD)
    wo4 = wo.reshape(NH, HD, D)

    in_maps = []
    for c in range(NCORES):
        b, g = divmod(c, TPG)
        qh = slice(g * NQL, (g + 1) * NQL)
        kvh = slice(g * NKVL, (g + 1) * NKVL)
        m = {
            "x": np.ascontiguousarray(x[b].reshape(S, D)),
            "wq": np.ascontiguousarray(
                wq4[:, qh, :][:, :, _PERM].reshape(D, NQL * HD)),
            "wk": np.ascontiguousarray(
                wk4[:, kvh, :][:, :, _PERM].reshape(D, NKVL * HD)),
            "wv": np.ascontiguousarray(wv4[:, kvh, :].reshape(D, NKVL * HD)),
            "wo": np.ascontiguousarray(wo4[qh].reshape(NQL * HD, D)),
            "cos": cos,
            "sin": sin,
        }
        in_maps.append(m)
    return in_maps


def kernel(x, start_pos, freqs_cos, freqs_sin, mask, wq, wk, wv, wo,
           cache_k, cache_v):
    x = np.asarray(x)
    in_maps = _shard_inputs(x, freqs_cos, freqs_sin, wq, wk, wv, wo)
    nc = _get_graph()
    res = run_bass_kernel_spmd(nc, in_maps, core_ids=list(range(NCORES)))
    out = np.zeros((B, S, D), dtype=np.float32)
    for b in range(B):
        acc = np.asarray(res.results[b * TPG]["out"], dtype=np.float32).copy()
        for g in range(1, TPG):
            acc += np.asarray(res.results[b * TPG + g]["out"], dtype=np.float32)
        out[b] = acc
    return out
